# revision 18
# baseline (speedup 1.0000x reference)
"""MoE BERT layer (nn_MoEBertLayer) on 8 Trainium2 NeuronCores.

Sharding: pure data parallel. B=8 samples -> 1 sample per core. The MoE
routing (per-sample expert selection) is done on the host: each core's
input map carries the weights of the expert its sample routed to, packed
into matmul-friendly tile layouts and pre-converted to bf16. The device
kernel is a dense BERT layer for a single [512, 768] sample.

v2 (bf16): all matmul operands are bf16 (same 1 cycle/row PE rate as
f32r, half the DMA bytes, and no f32r round-copy CASTs which burned
53us of DVE in v1). PSUM accumulation and LN statistics stay fp32.
Engine schedule is arranged to keep the PE queue dense so the tensor
engine stays at its 2.4GHz p-state:
  - QT/KT in [H,S] layout, V in [S,H]+ones-column layout (softmax
    denominator free via the attention-context matmul's last row).
  - attention head pairs: scores (K=64, PE quadrants 0/64), exp on ACT,
    ctx matmul, softmax normalization on the eviction path
    (gpsimd bcast + fast reciprocal).
  - Wo per s-tile with the k=5 (last head pair) term deferred to the
    end of each accumulation chain, hiding the last pair's
    normalization latency.
  - LN per s-tile (split stats: DVE accumulator for sum, ACT Square for
    sum of squares), then PE transposes x1 -> x1T for the FFN.
  - FFN1 streams Wi tiles; FFN2 runs s-tile-outer against a resident
    bf16 Wout so LN2 + output DMA of tile m overlap tile m+1 matmuls.
"""

import os
import sys
import numpy as np
import ml_dtypes
from contextlib import ExitStack

for _p in ("/opt/trn_rl_repo", os.path.expanduser("~/.axon_site/_ro/trn_rl_repo")):
    if os.path.isdir(_p) and _p not in sys.path:
        sys.path.append(_p)

import concourse.bass as bass
import concourse.bacc as bacc
import concourse.tile as tile
from concourse import mybir
from concourse.masks import make_identity

F32 = mybir.dt.float32
BF = mybir.dt.bfloat16
AF = mybir.ActivationFunctionType
NPBF = ml_dtypes.bfloat16

P = 128
S = 512           # sequence length (per sample)
H = 768           # hidden size
FF = 3072         # FFN intermediate
NH = 12           # attention heads
DH = 64           # head dim
HK = H // P       # 6
SQ = S // P       # 4
FK = FF // P      # 24
VW = DH + 1       # 65: V head block + ones column
N_CORES = 8
EPS = 1e-12


def _emit(ctx, tc, flags):
    nc = tc.nc
    (use_bq, use_bk, use_bv, use_bo, use_bi, use_bout,
     use_mask, use_ln1, use_ln2) = flags

    xT_d = nc.dram_tensor("xT", [H, S], BF, kind="ExternalInput")
    x_d = nc.dram_tensor("x", [S, H], BF, kind="ExternalInput")
    wq_d = nc.dram_tensor("wq", [HK, P, HK, P], BF, kind="ExternalInput")
    wk_d = nc.dram_tensor("wk", [HK, P, HK, P], BF, kind="ExternalInput")
    wv_d = nc.dram_tensor("wv", [HK, P, H], BF, kind="ExternalInput")
    wo_d = nc.dram_tensor("wo", [HK, P, H], BF, kind="ExternalInput")
    wi_d = nc.dram_tensor("wi", [FK, P, HK, P], BF, kind="ExternalInput")
    wout_d = nc.dram_tensor("wout", [FK, P, H], BF, kind="ExternalInput")
    out_d = nc.dram_tensor("out", [S, H], F32, kind="ExternalOutput")

    # optional inputs (general path; absent in the fast path)
    bq_d = nc.dram_tensor("bq", [P, HK], F32, kind="ExternalInput") if use_bq else None
    bk_d = nc.dram_tensor("bk", [P, HK], F32, kind="ExternalInput") if use_bk else None
    bv_d = nc.dram_tensor("bv", [H], F32, kind="ExternalInput") if use_bv else None
    bo_d = nc.dram_tensor("bo", [H], F32, kind="ExternalInput") if use_bo else None
    bi_d = nc.dram_tensor("bi", [P, FK], F32, kind="ExternalInput") if use_bi else None
    bout_d = nc.dram_tensor("bout", [H], F32, kind="ExternalInput") if use_bout else None
    msk_d = nc.dram_tensor("msk", [P, SQ], F32, kind="ExternalInput") if use_mask else None
    ln1g_d = nc.dram_tensor("ln1g", [H], F32, kind="ExternalInput") if use_ln1 else None
    ln1b_d = nc.dram_tensor("ln1b", [H], F32, kind="ExternalInput") if use_ln1 else None
    ln2g_d = nc.dram_tensor("ln2g", [H], F32, kind="ExternalInput") if use_ln2 else None
    ln2b_d = nc.dram_tensor("ln2b", [H], F32, kind="ExternalInput") if use_ln2 else None

    def bcast_dram_row(dram_ap, parts=P):
        # DRAM [N] -> partition-broadcast [parts, N] AP for DMA
        return bass.AP(tensor=dram_ap.tensor, offset=dram_ap.offset,
                       ap=[[0, parts]] + list(dram_ap.ap))

    # ---------------- pools: whole-kernel lifetime ----------------
    const = ctx.enter_context(tc.tile_pool(name="const", bufs=1))
    wres = ctx.enter_context(tc.tile_pool(name="wres", bufs=1))
    wstream = ctx.enter_context(tc.tile_pool(name="wstream", bufs=1))
    acts = ctx.enter_context(tc.tile_pool(name="acts", bufs=1))
    apool = ctx.enter_context(tc.tile_pool(name="apool", bufs=1))
    smalls = ctx.enter_context(tc.tile_pool(name="smalls", bufs=4))
    expp = ctx.enter_context(tc.tile_pool(name="expp", bufs=1))
    rbp = ctx.enter_context(tc.tile_pool(name="rbp", bufs=2))
    outp = ctx.enter_context(tc.tile_pool(name="outp", bufs=1))

    ident = const.tile([P, P], BF)
    make_identity(nc, ident)
    eps_t = const.tile([P, 1], F32)
    nc.vector.memset(eps_t, EPS)
    ones_t = const.tile([P, NH], BF)
    nc.vector.memset(ones_t, 1.0)

    bq_sb = bk_sb = bi_sb = None
    bv_bc = bo_bc = bout_bc = msk_sb = None
    ln1g_bc = ln1b_bc = ln2g_bc = ln2b_bc = None
    if use_bq:
        bq_sb = const.tile([P, HK], F32)
        nc.sync.dma_start(out=bq_sb, in_=bq_d[:])
    if use_bk:
        bk_sb = const.tile([P, HK], F32)
        nc.sync.dma_start(out=bk_sb, in_=bk_d[:])
    if use_bi:
        bi_sb = const.tile([P, FK], F32)
        nc.sync.dma_start(out=bi_sb, in_=bi_d[:])
    if use_bv:
        bv_bc = const.tile([P, H], F32)
        nc.sync.dma_start(out=bv_bc, in_=bcast_dram_row(bv_d[:]))
    if use_bo:
        bo_bc = const.tile([P, H], F32)
        nc.sync.dma_start(out=bo_bc, in_=bcast_dram_row(bo_d[:]))
    if use_bout:
        bout_bc = const.tile([P, H], F32)
        nc.sync.dma_start(out=bout_bc, in_=bcast_dram_row(bout_d[:]))
    if use_mask:
        msk_sb = const.tile([P, SQ], F32)
        nc.sync.dma_start(out=msk_sb, in_=msk_d[:])
    if use_ln1:
        ln1g_bc = const.tile([P, H], F32)
        nc.sync.dma_start(out=ln1g_bc, in_=bcast_dram_row(ln1g_d[:]))
        ln1b_bc = const.tile([P, H], F32)
        nc.sync.dma_start(out=ln1b_bc, in_=bcast_dram_row(ln1b_d[:]))
    if use_ln2:
        ln2g_bc = const.tile([P, H], F32)
        nc.sync.dma_start(out=ln2g_bc, in_=bcast_dram_row(ln2g_d[:]))
        ln2b_bc = const.tile([P, H], F32)
        nc.sync.dma_start(out=ln2b_bc, in_=bcast_dram_row(ln2b_d[:]))

    # ---------------- persistent activations / weights ----------------
    xTr_sb = acts.tile([P, HK, S], BF)
    x_sb = acts.tile([P, SQ, H], BF)
    qt_sb = acts.tile([P, HK, S], BF)
    kt_sb = acts.tile([P, HK, S], BF)
    vt_sb = acts.tile([P, SQ, NH * VW], BF)
    ctxt_sb = acts.tile([P, HK, S], BF)
    x1_sb = acts.tile([P, SQ, H], BF)
    x1t_sb = acts.tile([P, HK, S], BF)
    hmidt_sb = acts.tile([P, FK, S], BF)

    wq_t = wres.tile([P, HK, HK, P], BF)
    wk_t = wres.tile([P, HK, HK, P], BF)
    wv_t = wres.tile([P, HK, H], BF)
    wo_t = wres.tile([P, HK, H], BF)
    wout_t = wres.tile([P, FK, H], BF)

    # Input DMA prefetch, strictly in consumption order on one queue so
    # early phases never contend for HBM bandwidth with late-phase
    # weights. Multi-tile tensors move as a single rearranged DMA.
    nc.sync.dma_start(out=wq_t[:, 0, :, :], in_=wq_d[0])
    nc.sync.dma_start(out=xTr_sb, in_=xT_d[:].rearrange("(k p) s -> p k s", p=P))
    for m in range(1, HK):
        nc.sync.dma_start(out=wq_t[:, m, :, :], in_=wq_d[m])
    nc.sync.dma_start(out=wv_t, in_=wv_d[:].rearrange("k p h -> p k h"))
    for m in range(HK):
        nc.sync.dma_start(out=wk_t[:, m, :, :], in_=wk_d[m])
    nc.sync.dma_start(out=wo_t, in_=wo_d[:].rearrange("k p h -> p k h"))
    nc.sync.dma_start(out=x_sb, in_=x_d[:].rearrange("(m p) h -> p m h", p=P))
    nc.sync.dma_start(out=wout_t, in_=wout_d[:].rearrange("k p h -> p k h"))

    # residual + layer-norm for one [P, H] tile. Split stats: the
    # residual add computes sum(a) via the DVE accumulator, ACT's Square
    # computes sum(a^2); var = E[a^2] - mu^2.
    def resid_layernorm_tile(ps, resid, ebias, dst, g_bc, b_bc, use_gb,
                             dst_bf=None):
        a = apool.tile([P, H], F32, tag="a", bufs=4, name="a")
        asum = smalls.tile([P, 1], F32, tag="lnas", name="asum", bufs=4)
        if ebias is None:
            nc.vector.scalar_tensor_tensor(
                a, ps, 1.0, resid, mybir.AluOpType.mult,
                mybir.AluOpType.add, accum_out=asum)
        else:
            nc.vector.tensor_add(a, ps, resid)
            nc.vector.scalar_tensor_tensor(
                a, a, 1.0, ebias, mybir.AluOpType.mult,
                mybir.AluOpType.add, accum_out=asum)
        trash = smalls.tile([P, H], F32, tag="lntr", name="trash", bufs=1)
        sqsum = smalls.tile([P, 1], F32, tag="lnsq", name="sqsum", bufs=4)
        nc.scalar.activation(trash, a, AF.Square, accum_out=sqsum)
        mu = smalls.tile([P, 1], F32, tag="lnmu", name="mu", bufs=4)
        nc.vector.tensor_scalar_mul(mu, asum, 1.0 / H)
        var = smalls.tile([P, 1], F32, tag="lnvar", name="var", bufs=4)
        nc.vector.tensor_mul(var, mu, mu)
        nc.vector.scalar_tensor_tensor(
            var, sqsum, 1.0 / H, var, mybir.AluOpType.mult,
            mybir.AluOpType.subtract)
        sd = smalls.tile([P, 1], F32, tag="lnsd", name="sd", bufs=4)
        nc.scalar.activation(sd, var, AF.Sqrt, bias=eps_t)
        rsig = smalls.tile([P, 1], F32, tag="lnrs", name="rsig", bufs=4)
        nc.vector.reciprocal(rsig, sd)
        nm = smalls.tile([P, 1], F32, tag="lnnm", name="nm", bufs=4)
        nc.vector.tensor_scalar(nm, mu, rsig, -1.0,
                                mybir.AluOpType.mult, mybir.AluOpType.mult)
        if use_gb:
            # general path: apply in f32, then fold gamma/beta, then copy
            xf = apool.tile([P, H], F32, tag="xf", bufs=2, name="xf")
            nc.scalar.activation(xf, a, AF.Identity, bias=nm, scale=rsig)
            nc.vector.tensor_mul(xf, xf, g_bc)
            nc.vector.tensor_add(xf, xf, b_bc)
            nc.vector.tensor_copy(dst, xf)
        else:
            nc.scalar.activation(dst, a, AF.Identity, bias=nm, scale=rsig)

    # ================ phase A: QT/KT/V ================
    with ExitStack() as phase_a:
        psA = phase_a.enter_context(tc.tile_pool(name="psA", bufs=1, space="PSUM"))

        # ---- QT / KT:  out[m] = W[:, m-block]^T @ xT  ([H,S] layout) ----
        def qkt_tile(w_t, dst, b_sb, useb, m):
            ps = psA.tile([P, S], F32, tag="qk", bufs=3, name="psqk")
            for k in range(HK):
                nc.tensor.matmul(ps, lhsT=w_t[:, m, k, :],
                                 rhs=xTr_sb[:, k, :],
                                 start=(k == 0), stop=(k == HK - 1))
            if useb:
                nc.scalar.activation(dst[:, m, :], ps, AF.Identity,
                                     bias=b_sb[:, m:m + 1])
            else:
                # eviction on DVE: ACT must stay free for the exp chain
                nc.vector.tensor_copy(dst[:, m, :], ps)

        # ---- V in [S,H] layout with ones column per head -> vt_sb ----
        vt_v = vt_sb.rearrange("p m (h c) -> p m h c", c=VW)
        for m in range(SQ):
            nc.vector.tensor_copy(
                vt_v[:, m, :, DH:DH + 1],
                ones_t.rearrange("p (h o) -> p h o", o=1))

        def v_tile(m):
            ps = psA.tile([P, H], F32, tag="v", bufs=2, name="psv")
            for k in range(HK):
                nc.tensor.matmul(ps[:, 0:512],
                                 lhsT=xTr_sb[:, k, m * P:(m + 1) * P],
                                 rhs=wv_t[:, k, 0:512],
                                 start=(k == 0), stop=(k == HK - 1))
            for k in range(HK):
                nc.tensor.matmul(ps[:, 512:H],
                                 lhsT=xTr_sb[:, k, m * P:(m + 1) * P],
                                 rhs=wv_t[:, k, 512:H],
                                 start=(k == 0), stop=(k == HK - 1))
            src = ps.rearrange("p (h d) -> p h d", d=DH)
            if use_bv:
                nc.vector.tensor_add(
                    src, src, bv_bc.rearrange("p (h d) -> p h d", d=DH))
            nc.vector.tensor_copy(vt_v[:, m, :, 0:DH], src)

        # order QT -> V -> KT: the psA "v"-ring retires well before the
        # attention pools claim its banks, and the DVE eviction queue
        # (qt, vt, kt) never backs up against the scores gate.
        for m in range(HK):
            qkt_tile(wq_t, qt_sb, bq_sb, use_bq, m)
        for m in range(SQ):
            v_tile(m)
        for m in range(HK):
            qkt_tile(wk_t, kt_sb, bk_sb, use_bk, m)

    # ================ phase B: attention + Wo + LN1 + transpose ========
    with ExitStack() as phase_b:
        psB = phase_b.enter_context(tc.tile_pool(name="psB", bufs=1, space="PSUM"))

        # ---- attention, head pairs: heads 2hp/2hp+1 at partition bases
        # 0/64 of the kt/qt tiles -> score matmuls on PE quadrants; both
        # score outputs share one 2-bank psum tile so exp processes 1024
        # columns per ACT op. ----
        def scores_pair(hp):
            est_l = []
            for sk in range(SQ):
                ps_s = psB.tile([P, 2 * S], F32, tag="s", bufs=2, name="pss")
                for half in range(2):
                    pb = 64 * half
                    nc.tensor.matmul(
                        ps_s[:, half * S:(half + 1) * S],
                        lhsT=kt_sb[pb:pb + DH, hp, sk * P:(sk + 1) * P],
                        rhs=qt_sb[pb:pb + DH, hp, :],
                        start=True, stop=True)
                est = expp.tile([P, 2 * S], BF, tag="est", bufs=8, name="est")
                if use_mask:
                    nc.scalar.activation(est, ps_s, AF.Exp,
                                         bias=msk_sb[:, sk:sk + 1], scale=0.125)
                else:
                    nc.scalar.activation(est, ps_s, AF.Exp, scale=0.125)
                est_l.append(est)
            return est_l

        def ctx_pair(hp, est_l):
            # softmax normalization per half: sums-row gather + partition
            # broadcast ride the (otherwise idle) GpSimd, reciprocal +
            # fused normalize-eviction on DVE. Per-half chaining keeps
            # the post-matmul latency ~3us so the deferred Wo terms are
            # barely gated.
            for half in range(2):
                h = 2 * hp + half
                pb = 64 * half
                ps_c = psB.tile([P, S], F32, tag="c", bufs=4, name="psc")
                for sk in range(SQ):
                    nc.tensor.matmul(ps_c[0:VW, :],
                                     lhsT=vt_sb[:, sk, h * VW:(h + 1) * VW],
                                     rhs=est_l[sk][:, half * S:(half + 1) * S],
                                     start=(sk == 0), stop=(sk == SQ - 1))
                srow = smalls.tile([1, S], F32, tag="srow", bufs=4)
                nc.vector.tensor_copy(srow, ps_c[DH:VW, :])
                rb = rbp.tile([P, S], F32, tag="rb", bufs=2)
                nc.gpsimd.partition_broadcast(rb, srow)
                nc.vector.reciprocal_approx_fast(rb, rb)
                nc.vector.tensor_tensor(
                    ctxt_sb[pb:pb + DH, hp, :], ps_c[0:DH, :],
                    rb[pb:pb + DH, :],
                    mybir.AluOpType.mult)

        # software-pipelined: scores of pair hp+1 are emitted before ctx
        # of pair hp, so the PE never sits in-order behind the exp (ACT)
        # latency of the pair it is about to contract.
        est_prev = scores_pair(0)
        for hp in range(1, NH // 2):
            est_next = scores_pair(hp)
            ctx_pair(hp - 1, est_prev)
            est_prev = est_next
        ctx_pair(NH // 2 - 1, est_prev)

        # ---- Wo + residual + LN1, s-tile pipelined; the k=5 (last head
        # pair) matmul term runs at the end of each chain so the last
        # pair's normalization latency hides behind k=0..4 work. ----
        KORD = [0, 1, 2, 3, 4, 5]

        def wo_chain(m, korder):
            ps = psB.tile([P, 2 * S], F32, tag="s", bufs=2, name="psw")
            for i, k in enumerate(korder):
                nc.tensor.matmul(ps[:, 0:512],
                                 lhsT=ctxt_sb[:, k, m * P:(m + 1) * P],
                                 rhs=wo_t[:, k, 0:512],
                                 start=(i == 0), stop=(i == HK - 1))
            for i, k in enumerate(korder):
                nc.tensor.matmul(ps[:, 512:H],
                                 lhsT=ctxt_sb[:, k, m * P:(m + 1) * P],
                                 rhs=wo_t[:, k, 512:H],
                                 start=(i == 0), stop=(i == HK - 1))
            return ps[:, 0:H]

        def ln1_tile(m, ps):
            resid_layernorm_tile(ps, x_sb[:, m, :],
                                 bo_bc if use_bo else None,
                                 x1_sb[:, m, :], ln1g_bc, ln1b_bc, use_ln1)

        def transp_tile(m):
            # x1 -> x1T via PE transposes (bf16, 1 cycle/row); psum
            # evictions ride ACT (idle after attention) so DVE stays
            # free for the LN stat chains.
            for kb in range(HK):
                ps_t = psB.tile([P, P], BF, tag="c", bufs=4, name="pst")
                nc.tensor.transpose(
                    ps_t, x1_sb[:, m, kb * P:(kb + 1) * P], ident)
                nc.scalar.copy(
                    x1t_sb[:, kb, m * P:(m + 1) * P], ps_t)

        ps_w = {}
        # m0/m1 k0..4 first (hiding pair-5 latency), then their k5 terms
        for m in (0, 1):
            ps = psB.tile([P, 2 * S], F32, tag="s", bufs=2, name="psw")
            ps_w[m] = ps
            for half, lo, hi in ((0, 0, 512), (1, 512, H)):
                for i, k in enumerate(KORD[:5]):
                    nc.tensor.matmul(ps[:, lo:hi],
                                     lhsT=ctxt_sb[:, k, m * P:(m + 1) * P],
                                     rhs=wo_t[:, k, lo:hi],
                                     start=(i == 0), stop=False)
        for m in (0, 1):
            for half, lo, hi in ((0, 0, 512), (1, 512, H)):
                nc.tensor.matmul(ps_w[m][:, lo:hi],
                                 lhsT=ctxt_sb[:, 5, m * P:(m + 1) * P],
                                 rhs=wo_t[:, 5, lo:hi],
                                 start=False, stop=True)
        # emission order pipelines the four s-tiles across engines: the
        # PE runs Wo m2/m3 while ACT/DVE run LN chains of m0/m1, and the
        # m0/m1 transposes slot between the m2 and m3 matmul chains.
        ln1_tile(0, ps_w[0][:, 0:H])
        ln1_tile(1, ps_w[1][:, 0:H])
        ps2 = wo_chain(2, KORD)
        transp_tile(0)
        transp_tile(1)
        ps3 = wo_chain(3, KORD)
        ln1_tile(2, ps2)
        transp_tile(2)
        ln1_tile(3, ps3)
        transp_tile(3)

    # ================ phase C: FFN ================
    with ExitStack() as phase_c:
        psD = phase_c.enter_context(tc.tile_pool(name="psD", bufs=1, space="PSUM"))

        # ---- FFN1: hmidT[f,:] = Wi[:,f]^T @ x1T, GELU on eviction ----
        for mf in range(FK):
            wi_t = wstream.tile([P, HK, P], BF, tag="wi", bufs=6, name="wi")
            nc.sync.dma_start(out=wi_t, in_=wi_d[mf])
            ps = psD.tile([P, S], F32, tag="f1", bufs=4, name="psf1")
            for k in range(HK):
                nc.tensor.matmul(ps, lhsT=wi_t[:, k, :], rhs=x1t_sb[:, k, :],
                                 start=(k == 0), stop=(k == HK - 1))
            if use_bi:
                nc.scalar.activation(hmidt_sb[:, mf, :], ps, AF.Gelu,
                                     bias=bi_sb[:, mf:mf + 1])
            else:
                nc.scalar.activation(hmidt_sb[:, mf, :], ps, AF.Gelu)

        # ---- FFN2 s-tile-outer against resident Wout; LN2 + out DMA of
        # tile m overlap tile m+1 matmuls ----
        for m in range(SQ):
            ps = psD.tile([P, H], F32, tag="f2", bufs=2, name="psf2")
            for lo, hi in ((0, 512), (512, H)):
                for k in range(FK):
                    nc.tensor.matmul(ps[:, lo:hi],
                                     lhsT=hmidt_sb[:, k, m * P:(m + 1) * P],
                                     rhs=wout_t[:, k, lo:hi],
                                     start=(k == 0), stop=(k == FK - 1))
            o = outp.tile([P, H], F32, tag="out", bufs=2, name="o")
            resid_layernorm_tile(ps, x1_sb[:, m, :],
                                 bout_bc if use_bout else None,
                                 o, ln2g_bc, ln2b_bc, use_ln2)
            nc.sync.dma_start(out=out_d[m * P:(m + 1) * P, :], in_=o)


_NC_CACHE = {}


def build_nc(flags):
    key = tuple(flags)
    if key not in _NC_CACHE:
        nc = bacc.Bacc("TRN2")
        with ExitStack() as ctx:
            tc = ctx.enter_context(tile.TileContext(nc))
            _emit(ctx, tc, flags)
        nc.compile()
        _NC_CACHE[key] = nc
    return _NC_CACHE[key]


def _pack_lhsT(A, mt):
    # A [in, mt*P] -> [mt, P, in//P, P] tiles: out[m, p, k, f] = A[P*k+p, P*m+f]
    kt = A.shape[0] // P
    return np.ascontiguousarray(
        A.reshape(kt, P, mt, P).transpose(2, 1, 0, 3))


def _bf(a):
    return np.ascontiguousarray(np.asarray(a).astype(NPBF))


def kernel(**inputs):
    hs = np.ascontiguousarray(np.asarray(inputs["hidden_states"], dtype=np.float32))
    eidx = np.asarray(inputs["expert_idx"]).astype(np.int64)
    mask = np.asarray(inputs["attention_mask"], dtype=np.float32)
    Wq = np.asarray(inputs["Wq"], dtype=np.float32)
    bq = np.asarray(inputs["bq"], dtype=np.float32)
    Wk = np.asarray(inputs["Wk"], dtype=np.float32)
    bk = np.asarray(inputs["bk"], dtype=np.float32)
    Wv = np.asarray(inputs["Wv"], dtype=np.float32)
    bv = np.asarray(inputs["bv"], dtype=np.float32)
    Wo = np.asarray(inputs["Wo"], dtype=np.float32)
    bo = np.asarray(inputs["bo"], dtype=np.float32)
    ln1_g = np.asarray(inputs["ln1_g"], dtype=np.float32)
    ln1_b = np.asarray(inputs["ln1_b"], dtype=np.float32)
    Wi = np.asarray(inputs["Wi"], dtype=np.float32)
    bi = np.asarray(inputs["bi"], dtype=np.float32)
    Wout = np.asarray(inputs["Wout"], dtype=np.float32)
    bout = np.asarray(inputs["bout"], dtype=np.float32)
    ln2_g = np.asarray(inputs["ln2_g"], dtype=np.float32)
    ln2_b = np.asarray(inputs["ln2_b"], dtype=np.float32)

    B = hs.shape[0]
    assert hs.shape == (B, S, H) and B == N_CORES

    use_bq = bool(np.any(bq))
    use_bk = bool(np.any(bk))
    use_bv = bool(np.any(bv))
    use_bo = bool(np.any(bo))
    use_bi = bool(np.any(bi))
    use_bout = bool(np.any(bout))
    use_mask = bool(np.any(mask))
    use_ln1 = bool(np.any(ln1_g != 1.0) or np.any(ln1_b))
    use_ln2 = bool(np.any(ln2_g != 1.0) or np.any(ln2_b))
    flags = (use_bq, use_bk, use_bv, use_bo, use_bi, use_bout,
             use_mask, use_ln1, use_ln2)

    nc = build_nc(flags)

    # per-expert packed weights, converted once and reused across cores
    packed = {}
    for e in set(int(v) for v in eidx):
        packed[e] = {
            "wq": _bf(_pack_lhsT(Wq[e], HK)),
            "wk": _bf(_pack_lhsT(Wk[e], HK)),
            "wv": _bf(Wv[e].reshape(HK, P, H)),
            "wo": _bf(Wo[e].reshape(HK, P, H)),
            "wi": _bf(_pack_lhsT(Wi[e], FK)),
            "wout": _bf(Wout[e].reshape(FK, P, H)),
        }

    in_maps = []
    for b in range(B):
        e = int(eidx[b])
        xb = hs[b]
        im = {
            "x": _bf(xb),
            "xT": _bf(xb.T),
        }
        im.update(packed[e])
        if use_bq:
            im["bq"] = np.ascontiguousarray(bq[e].reshape(HK, P).T)
        if use_bk:
            im["bk"] = np.ascontiguousarray(bk[e].reshape(HK, P).T)
        if use_bv:
            im["bv"] = bv[e]
        if use_bo:
            im["bo"] = bo[e]
        if use_bi:
            im["bi"] = np.ascontiguousarray(bi[e].reshape(FK, P).T)
        if use_bout:
            im["bout"] = bout[e]
        if use_mask:
            im["msk"] = np.ascontiguousarray(mask[b, 0, 0, :].reshape(SQ, P).T)
        if use_ln1:
            im["ln1g"] = ln1_g
            im["ln1b"] = ln1_b
        if use_ln2:
            im["ln2g"] = ln2_g
            im["ln2b"] = ln2_b
        in_maps.append(im)

    from concourse.bass_utils import run_bass_kernel_spmd
    res = run_bass_kernel_spmd(nc, in_maps, core_ids=list(range(N_CORES)),
                               **RUN_KWARGS)
    global LAST_RESULTS
    LAST_RESULTS = res
    out = np.stack([res.results[b]["out"] for b in range(B)], axis=0)
    return out.astype(np.float32)


RUN_KWARGS = {}
LAST_RESULTS = None


if __name__ == "__main__":
    rng = np.random.default_rng(0)
    demo = {
        "hidden_states": rng.standard_normal((8, S, H), dtype=np.float32),
        "expert_idx": rng.integers(0, 4, size=8).astype(np.int32),
        "attention_mask": np.zeros((8, 1, 1, S), np.float32),
        "Wq": 0.02 * rng.standard_normal((4, H, H), dtype=np.float32),
        "bq": np.zeros((4, H), np.float32),
        "Wk": 0.02 * rng.standard_normal((4, H, H), dtype=np.float32),
        "bk": np.zeros((4, H), np.float32),
        "Wv": 0.02 * rng.standard_normal((4, H, H), dtype=np.float32),
        "bv": np.zeros((4, H), np.float32),
        "Wo": 0.02 * rng.standard_normal((4, H, H), dtype=np.float32),
        "bo": np.zeros((4, H), np.float32),
        "ln1_g": np.ones((H,), np.float32),
        "ln1_b": np.zeros((H,), np.float32),
        "Wi": 0.02 * rng.standard_normal((4, H, FF), dtype=np.float32),
        "bi": np.zeros((4, FF), np.float32),
        "Wout": 0.02 * rng.standard_normal((4, FF, H), dtype=np.float32),
        "bout": np.zeros((4, H), np.float32),
        "ln2_g": np.ones((H,), np.float32),
        "ln2_b": np.zeros((H,), np.float32),
    }
    out = kernel(**demo)
    print("out", out.shape, out.dtype, float(np.abs(out).mean()))


# revision 31
# speedup vs baseline: 1.2485x; 1.2485x over previous
"""MoE BERT layer (nn_MoEBertLayer) on 8 Trainium2 NeuronCores.

Sharding: pure data parallel. B=8 samples -> 1 sample per core. The MoE
routing (per-sample expert selection) is done on the host: each core's
input map carries the weights of the expert its sample routed to, packed
into matmul-friendly tile layouts and pre-converted to bf16. The device
kernel is a dense BERT layer for a single [512, 768] sample.

v2 (bf16): all matmul operands are bf16 (same 1 cycle/row PE rate as
f32r, half the DMA bytes, and no f32r round-copy CASTs which burned
53us of DVE in v1). PSUM accumulation and LN statistics stay fp32.
Engine schedule is arranged to keep the PE queue dense so the tensor
engine stays at its 2.4GHz p-state:
  - QT/KT in [H,S] layout, V in [S,H]+ones-column layout (softmax
    denominator free via the attention-context matmul's last row).
  - attention head pairs: scores (K=64, PE quadrants 0/64), exp on ACT,
    ctx matmul, softmax normalization on the eviction path
    (gpsimd bcast + fast reciprocal).
  - Wo per s-tile with the k=5 (last head pair) term deferred to the
    end of each accumulation chain, hiding the last pair's
    normalization latency.
  - LN per s-tile (split stats: DVE accumulator for sum, ACT Square for
    sum of squares), then PE transposes x1 -> x1T for the FFN.
  - FFN1 streams Wi tiles; FFN2 runs s-tile-outer against a resident
    bf16 Wout so LN2 + output DMA of tile m overlap tile m+1 matmuls.
"""

import os
import sys
import numpy as np
import ml_dtypes
from contextlib import ExitStack

for _p in ("/opt/trn_rl_repo", os.path.expanduser("~/.axon_site/_ro/trn_rl_repo")):
    if os.path.isdir(_p) and _p not in sys.path:
        sys.path.append(_p)

import concourse.bass as bass
import concourse.bacc as bacc
import concourse.tile as tile
from concourse import mybir
from concourse.masks import make_identity

F32 = mybir.dt.float32
BF = mybir.dt.bfloat16
F8 = mybir.dt.float8e4
DR = mybir.MatmulPerfMode.DoubleRow
AF = mybir.ActivationFunctionType
NPBF = ml_dtypes.bfloat16
NPF8 = ml_dtypes.float8_e4m3
F8MAX = 240.0     # e4m3 max finite (overflows to inf beyond)
CTXS = 128.0      # fp8 pre-scale for the (small-magnitude) ctx values

P = 128
S = 512           # sequence length (per sample)
H = 768           # hidden size
FF = 3072         # FFN intermediate
NH = 12           # attention heads
DH = 64           # head dim
HK = H // P       # 6
SQ = S // P       # 4
FK = FF // P      # 24
VW = DH + 1       # 65: V head block + ones column
N_CORES = 8
EPS = 1e-12


def _emit(ctx, tc, flags):
    nc = tc.nc
    (use_bq, use_bk, use_bv, use_bo, use_bi, use_bout,
     use_mask, use_ln1, use_ln2) = flags

    xT_d = nc.dram_tensor("xT", [H, S], F8, kind="ExternalInput")
    x_d = nc.dram_tensor("x", [S, H], BF, kind="ExternalInput")
    wq_d = nc.dram_tensor("wq", [HK, P, HK, P], F8, kind="ExternalInput")
    wk_d = nc.dram_tensor("wk", [HK, P, HK, P], F8, kind="ExternalInput")
    wv_d = nc.dram_tensor("wv", [HK, P, H], F8, kind="ExternalInput")
    wo_d = nc.dram_tensor("wo", [HK, P, H], F8, kind="ExternalInput")
    wi_d = nc.dram_tensor("wi", [FK, P, HK, P], BF, kind="ExternalInput")
    wout_d = nc.dram_tensor("wout", [FK, P, H], BF, kind="ExternalInput")
    scl_d = nc.dram_tensor("scl", [4], F32, kind="ExternalInput")
    out_d = nc.dram_tensor("out", [S, H], F32, kind="ExternalOutput")

    # optional inputs (general path; absent in the fast path)
    bq_d = nc.dram_tensor("bq", [P, HK], F32, kind="ExternalInput") if use_bq else None
    bk_d = nc.dram_tensor("bk", [P, HK], F32, kind="ExternalInput") if use_bk else None
    bv_d = nc.dram_tensor("bv", [H], F32, kind="ExternalInput") if use_bv else None
    bo_d = nc.dram_tensor("bo", [H], F32, kind="ExternalInput") if use_bo else None
    bi_d = nc.dram_tensor("bi", [P, FK], F32, kind="ExternalInput") if use_bi else None
    bout_d = nc.dram_tensor("bout", [H], F32, kind="ExternalInput") if use_bout else None
    msk_d = nc.dram_tensor("msk", [P, SQ], F32, kind="ExternalInput") if use_mask else None
    ln1g_d = nc.dram_tensor("ln1g", [H], F32, kind="ExternalInput") if use_ln1 else None
    ln1b_d = nc.dram_tensor("ln1b", [H], F32, kind="ExternalInput") if use_ln1 else None
    ln2g_d = nc.dram_tensor("ln2g", [H], F32, kind="ExternalInput") if use_ln2 else None
    ln2b_d = nc.dram_tensor("ln2b", [H], F32, kind="ExternalInput") if use_ln2 else None

    def bcast_dram_row(dram_ap, parts=P):
        # DRAM [N] -> partition-broadcast [parts, N] AP for DMA
        return bass.AP(tensor=dram_ap.tensor, offset=dram_ap.offset,
                       ap=[[0, parts]] + list(dram_ap.ap))

    # ---------------- pools: whole-kernel lifetime ----------------
    const = ctx.enter_context(tc.tile_pool(name="const", bufs=1))
    wres = ctx.enter_context(tc.tile_pool(name="wres", bufs=1))
    wstream = ctx.enter_context(tc.tile_pool(name="wstream", bufs=1))
    acts = ctx.enter_context(tc.tile_pool(name="acts", bufs=1))
    apool = ctx.enter_context(tc.tile_pool(name="apool", bufs=1))
    smalls = ctx.enter_context(tc.tile_pool(name="smalls", bufs=4))
    expp = ctx.enter_context(tc.tile_pool(name="expp", bufs=1))
    rbp = ctx.enter_context(tc.tile_pool(name="rbp", bufs=2))
    outp = ctx.enter_context(tc.tile_pool(name="outp", bufs=1))

    ident = const.tile([P, P], BF)
    make_identity(nc, ident)
    eps_t = const.tile([P, 1], F32)
    nc.vector.memset(eps_t, EPS)
    ones_t = const.tile([P, NH], BF)
    nc.vector.memset(ones_t, 1.0)
    # fp8 dequant scales (per expert/sample, computed on host):
    # col 0: qkT evict (s_x*s_wq), 1: kT (s_x*s_wk), 2: v (s_x*s_wv),
    # col 3: wo psum (s_wo/CTXS)
    scl_sb = const.tile([P, 4], F32)
    nc.sync.dma_start(out=scl_sb, in_=bcast_dram_row(scl_d[:]))

    bq_sb = bk_sb = bi_sb = None
    bv_bc = bo_bc = bout_bc = msk_sb = None
    ln1g_bc = ln1b_bc = ln2g_bc = ln2b_bc = None
    if use_bq:
        bq_sb = const.tile([P, HK], F32)
        nc.sync.dma_start(out=bq_sb, in_=bq_d[:])
    if use_bk:
        bk_sb = const.tile([P, HK], F32)
        nc.sync.dma_start(out=bk_sb, in_=bk_d[:])
    if use_bi:
        bi_sb = const.tile([P, FK], F32)
        nc.sync.dma_start(out=bi_sb, in_=bi_d[:])
    if use_bv:
        bv_bc = const.tile([P, H], F32)
        nc.sync.dma_start(out=bv_bc, in_=bcast_dram_row(bv_d[:]))
    if use_bo:
        bo_bc = const.tile([P, H], F32)
        nc.sync.dma_start(out=bo_bc, in_=bcast_dram_row(bo_d[:]))
    if use_bout:
        bout_bc = const.tile([P, H], F32)
        nc.sync.dma_start(out=bout_bc, in_=bcast_dram_row(bout_d[:]))
    if use_mask:
        msk_sb = const.tile([P, SQ], F32)
        nc.sync.dma_start(out=msk_sb, in_=msk_d[:])
    if use_ln1:
        ln1g_bc = const.tile([P, H], F32)
        nc.sync.dma_start(out=ln1g_bc, in_=bcast_dram_row(ln1g_d[:]))
        ln1b_bc = const.tile([P, H], F32)
        nc.sync.dma_start(out=ln1b_bc, in_=bcast_dram_row(ln1b_d[:]))
    if use_ln2:
        ln2g_bc = const.tile([P, H], F32)
        nc.sync.dma_start(out=ln2g_bc, in_=bcast_dram_row(ln2g_d[:]))
        ln2b_bc = const.tile([P, H], F32)
        nc.sync.dma_start(out=ln2b_bc, in_=bcast_dram_row(ln2b_d[:]))

    # ---------------- persistent activations / weights ----------------
    xTr_sb = acts.tile([P, HK, S], F8)
    x_sb = acts.tile([P, SQ, H], BF)
    qt_sb = acts.tile([P, HK, S], BF)
    kt_sb = acts.tile([P, HK, S], BF)
    vt_sb = acts.tile([P, SQ, NH * VW], BF)
    ctxt_sb = acts.tile([P, HK, S], F8)
    x1_sb = acts.tile([P, SQ, H], BF)
    x1t_sb = acts.tile([P, HK, S], BF)
    hmidt_sb = acts.tile([P, FK, S], BF)

    wq_t = wres.tile([P, HK, HK, P], F8)
    wk_t = wres.tile([P, HK, HK, P], F8)
    wv_t = wres.tile([P, HK, H], F8)
    wo_t = wres.tile([P, HK, H], F8)
    wout_t = wres.tile([P, FK, H], BF)

    # Input DMA prefetch, strictly in consumption order on one queue so
    # early phases never contend for HBM bandwidth with late-phase
    # weights. Multi-tile tensors move as a single rearranged DMA.
    nc.sync.dma_start(out=wq_t[:, 0, :, :], in_=wq_d[0])
    nc.sync.dma_start(out=xTr_sb, in_=xT_d[:].rearrange("(k p) s -> p k s", p=P))
    for m in range(1, HK):
        nc.sync.dma_start(out=wq_t[:, m, :, :], in_=wq_d[m])
    nc.sync.dma_start(out=wv_t, in_=wv_d[:].rearrange("k p h -> p k h"))
    for m in range(HK):
        nc.sync.dma_start(out=wk_t[:, m, :, :], in_=wk_d[m])
    nc.sync.dma_start(out=wo_t, in_=wo_d[:].rearrange("k p h -> p k h"))
    nc.sync.dma_start(out=x_sb, in_=x_d[:].rearrange("(m p) h -> p m h", p=P))
    nc.sync.dma_start(out=wout_t, in_=wout_d[:].rearrange("k p h -> p k h"))

    # residual + layer-norm for one [P, H] tile. Split stats: the
    # residual add computes sum(a) via the DVE accumulator, ACT's Square
    # computes sum(a^2); var = E[a^2] - mu^2.
    def resid_layernorm_tile(ps, resid, ebias, dst, g_bc, b_bc, use_gb,
                             pscale=1.0):
        a = apool.tile([P, H], F32, tag="a", bufs=4, name="a")
        asum = smalls.tile([P, 1], F32, tag="lnas", name="asum", bufs=4)
        if ebias is None:
            nc.vector.scalar_tensor_tensor(
                a, ps, pscale, resid, mybir.AluOpType.mult,
                mybir.AluOpType.add, accum_out=asum)
        else:
            nc.vector.scalar_tensor_tensor(
                a, ps, pscale, ebias, mybir.AluOpType.mult,
                mybir.AluOpType.add)
            nc.vector.scalar_tensor_tensor(
                a, a, 1.0, resid, mybir.AluOpType.mult,
                mybir.AluOpType.add, accum_out=asum)
        trash = smalls.tile([P, H], F32, tag="lntr", name="trash", bufs=1)
        sqsum = smalls.tile([P, 1], F32, tag="lnsq", name="sqsum", bufs=4)
        nc.scalar.activation(trash, a, AF.Square, accum_out=sqsum)
        mu = smalls.tile([P, 1], F32, tag="lnmu", name="mu", bufs=4)
        nc.vector.tensor_scalar_mul(mu, asum, 1.0 / H)
        var = smalls.tile([P, 1], F32, tag="lnvar", name="var", bufs=4)
        nc.vector.tensor_mul(var, mu, mu)
        nc.vector.scalar_tensor_tensor(
            var, sqsum, 1.0 / H, var, mybir.AluOpType.mult,
            mybir.AluOpType.subtract)
        sd = smalls.tile([P, 1], F32, tag="lnsd", name="sd", bufs=4)
        nc.scalar.activation(sd, var, AF.Sqrt, bias=eps_t)
        rsig = smalls.tile([P, 1], F32, tag="lnrs", name="rsig", bufs=4)
        nc.vector.reciprocal(rsig, sd)
        nm = smalls.tile([P, 1], F32, tag="lnnm", name="nm", bufs=4)
        nc.vector.tensor_scalar(nm, mu, rsig, -1.0,
                                mybir.AluOpType.mult, mybir.AluOpType.mult)
        if use_gb:
            # general path: apply in f32, then fold gamma/beta, then copy
            xf = apool.tile([P, H], F32, tag="xf", bufs=2, name="xf")
            nc.scalar.activation(xf, a, AF.Identity, bias=nm, scale=rsig)
            nc.vector.tensor_mul(xf, xf, g_bc)
            nc.vector.tensor_add(xf, xf, b_bc)
            nc.vector.tensor_copy(dst, xf)
        else:
            nc.scalar.activation(dst, a, AF.Identity, bias=nm, scale=rsig)

    # ================ phase A: QT/KT/V ================
    with ExitStack() as phase_a:
        psA = phase_a.enter_context(tc.tile_pool(name="psA", bufs=1, space="PSUM"))

        # ---- QT / KT:  out[m] = W[:, m-block]^T @ xT  ([H,S] layout) ----
        # fp8 DoubleRow: each matmul contracts a k-tile PAIR (the [:, 2j:
        # 2j+2, :] slice), 0.5 cycles/row; the fp8 dequant scale folds
        # into the psum eviction.
        def qkt_tile(w_t, dst, b_sb, useb, m, ci):
            ps = psA.tile([P, S], F32, tag="qk", bufs=3, name="psqk")
            for j in range(HK // 2):
                nc.tensor.matmul(ps, lhsT=w_t[:, m, 2 * j:2 * j + 2, :],
                                 rhs=xTr_sb[:, 2 * j:2 * j + 2, :],
                                 start=(j == 0), stop=(j == HK // 2 - 1),
                                 perf_mode=DR)
            if useb:
                nc.scalar.activation(dst[:, m, :], ps, AF.Identity,
                                     bias=b_sb[:, m:m + 1],
                                     scale=scl_sb[:, ci:ci + 1])
            else:
                # eviction on DVE: ACT must stay free for the exp chain
                nc.vector.tensor_scalar_mul(dst[:, m, :], ps,
                                            scl_sb[:, ci:ci + 1])

        # ---- V in [S,H] layout with ones column per head -> vt_sb ----
        vt_v = vt_sb.rearrange("p m (h c) -> p m h c", c=VW)
        for m in range(SQ):
            nc.vector.tensor_copy(
                vt_v[:, m, :, DH:DH + 1],
                ones_t.rearrange("p (h o) -> p h o", o=1))

        def v_tile(m):
            ps = psA.tile([P, H], F32, tag="v", bufs=2, name="psv")
            for lo, hi in ((0, 512), (512, H)):
                for j in range(HK // 2):
                    nc.tensor.matmul(ps[:, lo:hi],
                                     lhsT=xTr_sb[:, 2 * j:2 * j + 2,
                                                 m * P:(m + 1) * P],
                                     rhs=wv_t[:, 2 * j:2 * j + 2, lo:hi],
                                     start=(j == 0), stop=(j == HK // 2 - 1),
                                     perf_mode=DR)
            src = ps.rearrange("p (h d) -> p h d", d=DH)
            if use_bv:
                nc.vector.scalar_tensor_tensor(
                    vt_v[:, m, :, 0:DH], src, scl_sb[:, 2:3],
                    bv_bc.rearrange("p (h d) -> p h d", d=DH),
                    mybir.AluOpType.mult, mybir.AluOpType.add)
            else:
                nc.vector.tensor_scalar_mul(vt_v[:, m, :, 0:DH], src,
                                            scl_sb[:, 2:3])

        # order QT -> V -> KT: the psA "v"-ring retires well before the
        # attention pools claim its banks, and the DVE eviction queue
        # (qt, vt, kt) never backs up against the scores gate.
        for m in range(HK):
            qkt_tile(wq_t, qt_sb, bq_sb, use_bq, m, 0)
        for m in range(SQ):
            v_tile(m)
        for m in range(HK):
            qkt_tile(wk_t, kt_sb, bk_sb, use_bk, m, 1)

    # ================ phase B: attention + Wo + LN1 + transpose ========
    with ExitStack() as phase_b:
        psB = phase_b.enter_context(tc.tile_pool(name="psB", bufs=1, space="PSUM"))

        # ---- attention, head pairs: heads 2hp/2hp+1 at partition bases
        # 0/64 of the kt/qt tiles -> score matmuls on PE quadrants; both
        # score outputs share one 2-bank psum tile so exp processes 1024
        # columns per ACT op. ----
        def scores_pair(hp):
            est_l = []
            for sk in range(SQ):
                ps_s = psB.tile([P, 2 * S], F32, tag="s", bufs=2, name="pss")
                for half in range(2):
                    pb = 64 * half
                    nc.tensor.matmul(
                        ps_s[:, half * S:(half + 1) * S],
                        lhsT=kt_sb[pb:pb + DH, hp, sk * P:(sk + 1) * P],
                        rhs=qt_sb[pb:pb + DH, hp, :],
                        start=True, stop=True)
                est = expp.tile([P, 2 * S], BF, tag="est", bufs=8, name="est")
                if use_mask:
                    nc.scalar.activation(est, ps_s, AF.Exp,
                                         bias=msk_sb[:, sk:sk + 1], scale=0.125)
                else:
                    nc.scalar.activation(est, ps_s, AF.Exp, scale=0.125)
                est_l.append(est)
            return est_l

        def ctx_pair(hp, est_l):
            # softmax normalization per half: sums-row gather + partition
            # broadcast ride the (otherwise idle) GpSimd, reciprocal +
            # fused normalize-eviction on DVE. Per-half chaining keeps
            # the post-matmul latency ~3us so the deferred Wo terms are
            # barely gated.
            for half in range(2):
                h = 2 * hp + half
                pb = 64 * half
                ps_c = psB.tile([P, S], F32, tag="c", bufs=4, name="psc")
                for sk in range(SQ):
                    nc.tensor.matmul(ps_c[0:VW, :],
                                     lhsT=vt_sb[:, sk, h * VW:(h + 1) * VW],
                                     rhs=est_l[sk][:, half * S:(half + 1) * S],
                                     start=(sk == 0), stop=(sk == SQ - 1))
                srow = smalls.tile([1, S], F32, tag="srow", bufs=4)
                nc.vector.tensor_copy(srow, ps_c[DH:VW, :])
                rb = rbp.tile([P, S], F32, tag="rb", bufs=2)
                nc.gpsimd.partition_broadcast(rb, srow)
                nc.vector.reciprocal_approx_fast(rb, rb)
                # normalize + CTXS fp8 pre-scale fused into the eviction
                nc.vector.scalar_tensor_tensor(
                    ctxt_sb[pb:pb + DH, hp, :], ps_c[0:DH, :], CTXS,
                    rb[pb:pb + DH, :],
                    mybir.AluOpType.mult, mybir.AluOpType.mult)

        # software-pipelined: scores of pair hp+1 are emitted before ctx
        # of pair hp, so the PE never sits in-order behind the exp (ACT)
        # latency of the pair it is about to contract.
        est_prev = scores_pair(0)
        for hp in range(1, NH // 2):
            est_next = scores_pair(hp)
            ctx_pair(hp - 1, est_prev)
            est_prev = est_next
        ctx_pair(NH // 2 - 1, est_prev)

        # ---- Wo + residual + LN1, s-tile pipelined; the j=2 DoubleRow
        # term (head pairs 4+5) runs at the end of each chain so the
        # last pairs' normalization latency hides behind earlier work. ----
        def wo_chain(m):
            ps = psB.tile([P, 2 * S], F32, tag="s", bufs=2, name="psw")
            for lo, hi in ((0, 512), (512, H)):
                for j in range(HK // 2):
                    nc.tensor.matmul(ps[:, lo:hi],
                                     lhsT=ctxt_sb[:, 2 * j:2 * j + 2,
                                                 m * P:(m + 1) * P],
                                     rhs=wo_t[:, 2 * j:2 * j + 2, lo:hi],
                                     start=(j == 0), stop=(j == HK // 2 - 1),
                                     perf_mode=DR)
            return ps[:, 0:H]

        def ln1_tile(m, ps):
            resid_layernorm_tile(ps, x_sb[:, m, :],
                                 bo_bc if use_bo else None,
                                 x1_sb[:, m, :], ln1g_bc, ln1b_bc, use_ln1,
                                 pscale=scl_sb[:, 3:4])

        def transp_tile(m):
            # x1 -> x1T via PE transposes (bf16, 1 cycle/row); psum
            # evictions ride ACT (idle after attention) so DVE stays
            # free for the LN stat chains.
            for kb in range(HK):
                ps_t = psB.tile([P, P], BF, tag="c", bufs=4, name="pst")
                nc.tensor.transpose(
                    ps_t, x1_sb[:, m, kb * P:(kb + 1) * P], ident)
                nc.scalar.copy(
                    x1t_sb[:, kb, m * P:(m + 1) * P], ps_t)

        ps_w = {}
        # m0/m1 j0/j1 first (hiding the last pairs' eviction latency),
        # then their j2 terms
        for m in (0, 1):
            ps = psB.tile([P, 2 * S], F32, tag="s", bufs=2, name="psw")
            ps_w[m] = ps
            for half, lo, hi in ((0, 0, 512), (1, 512, H)):
                for j in (0, 1):
                    nc.tensor.matmul(ps[:, lo:hi],
                                     lhsT=ctxt_sb[:, 2 * j:2 * j + 2,
                                                 m * P:(m + 1) * P],
                                     rhs=wo_t[:, 2 * j:2 * j + 2, lo:hi],
                                     start=(j == 0), stop=False,
                                     perf_mode=DR)
        for m in (0, 1):
            for half, lo, hi in ((0, 0, 512), (1, 512, H)):
                nc.tensor.matmul(ps_w[m][:, lo:hi],
                                 lhsT=ctxt_sb[:, 4:6, m * P:(m + 1) * P],
                                 rhs=wo_t[:, 4:6, lo:hi],
                                 start=False, stop=True, perf_mode=DR)
        # emission order pipelines the four s-tiles across engines: the
        # PE runs Wo m2/m3 while ACT/DVE run LN chains of m0/m1, and the
        # m0/m1 transposes slot between the m2 and m3 matmul chains.
        ln1_tile(0, ps_w[0][:, 0:H])
        ln1_tile(1, ps_w[1][:, 0:H])
        ps2 = wo_chain(2)
        transp_tile(0)
        transp_tile(1)
        ps3 = wo_chain(3)
        ln1_tile(2, ps2)
        transp_tile(2)
        ln1_tile(3, ps3)
        transp_tile(3)

    # ================ phase C: FFN ================
    with ExitStack() as phase_c:
        psD = phase_c.enter_context(tc.tile_pool(name="psD", bufs=1, space="PSUM"))

        # ---- FFN1: hmidT[f,:] = Wi[:,f]^T @ x1T, GELU on eviction ----
        for mf in range(FK):
            wi_t = wstream.tile([P, HK, P], BF, tag="wi", bufs=6, name="wi")
            nc.sync.dma_start(out=wi_t, in_=wi_d[mf])
            ps = psD.tile([P, S], F32, tag="f1", bufs=4, name="psf1")
            for k in range(HK):
                nc.tensor.matmul(ps, lhsT=wi_t[:, k, :], rhs=x1t_sb[:, k, :],
                                 start=(k == 0), stop=(k == HK - 1))
            if use_bi:
                nc.scalar.activation(hmidt_sb[:, mf, :], ps, AF.Gelu,
                                     bias=bi_sb[:, mf:mf + 1])
            else:
                nc.scalar.activation(hmidt_sb[:, mf, :], ps, AF.Gelu)

        # ---- FFN2 s-tile-outer against resident Wout; LN2 + out DMA of
        # tile m overlap tile m+1 matmuls ----
        for m in range(SQ):
            ps = psD.tile([P, H], F32, tag="f2", bufs=2, name="psf2")
            for lo, hi in ((0, 512), (512, H)):
                for k in range(FK):
                    nc.tensor.matmul(ps[:, lo:hi],
                                     lhsT=hmidt_sb[:, k, m * P:(m + 1) * P],
                                     rhs=wout_t[:, k, lo:hi],
                                     start=(k == 0), stop=(k == FK - 1))
            o = outp.tile([P, H], F32, tag="out", bufs=2, name="o")
            resid_layernorm_tile(ps, x1_sb[:, m, :],
                                 bout_bc if use_bout else None,
                                 o, ln2g_bc, ln2b_bc, use_ln2)
            nc.sync.dma_start(out=out_d[m * P:(m + 1) * P, :], in_=o)


_NC_CACHE = {}


def build_nc(flags):
    key = tuple(flags)
    if key not in _NC_CACHE:
        nc = bacc.Bacc("TRN2")
        with ExitStack() as ctx:
            tc = ctx.enter_context(tile.TileContext(nc))
            _emit(ctx, tc, flags)
        nc.compile()
        _NC_CACHE[key] = nc
    return _NC_CACHE[key]


def _pack_lhsT(A, mt):
    # A [in, mt*P] -> [mt, P, in//P, P] tiles: out[m, p, k, f] = A[P*k+p, P*m+f]
    kt = A.shape[0] // P
    return np.ascontiguousarray(
        A.reshape(kt, P, mt, P).transpose(2, 1, 0, 3))


def _bf(a):
    return np.ascontiguousarray(np.asarray(a).astype(NPBF))


def _absmax(a):
    m = float(np.max(np.abs(a)))
    return m if m > 0 else 1.0


def _f8(a, s):
    return np.ascontiguousarray(
        np.clip(np.asarray(a, np.float32) * (1.0 / s), -F8MAX, F8MAX)
        .astype(NPF8))


def kernel(**inputs):
    hs = np.ascontiguousarray(np.asarray(inputs["hidden_states"], dtype=np.float32))
    eidx = np.asarray(inputs["expert_idx"]).astype(np.int64)
    mask = np.asarray(inputs["attention_mask"], dtype=np.float32)
    Wq = np.asarray(inputs["Wq"], dtype=np.float32)
    bq = np.asarray(inputs["bq"], dtype=np.float32)
    Wk = np.asarray(inputs["Wk"], dtype=np.float32)
    bk = np.asarray(inputs["bk"], dtype=np.float32)
    Wv = np.asarray(inputs["Wv"], dtype=np.float32)
    bv = np.asarray(inputs["bv"], dtype=np.float32)
    Wo = np.asarray(inputs["Wo"], dtype=np.float32)
    bo = np.asarray(inputs["bo"], dtype=np.float32)
    ln1_g = np.asarray(inputs["ln1_g"], dtype=np.float32)
    ln1_b = np.asarray(inputs["ln1_b"], dtype=np.float32)
    Wi = np.asarray(inputs["Wi"], dtype=np.float32)
    bi = np.asarray(inputs["bi"], dtype=np.float32)
    Wout = np.asarray(inputs["Wout"], dtype=np.float32)
    bout = np.asarray(inputs["bout"], dtype=np.float32)
    ln2_g = np.asarray(inputs["ln2_g"], dtype=np.float32)
    ln2_b = np.asarray(inputs["ln2_b"], dtype=np.float32)

    B = hs.shape[0]
    assert hs.shape == (B, S, H) and B == N_CORES

    use_bq = bool(np.any(bq))
    use_bk = bool(np.any(bk))
    use_bv = bool(np.any(bv))
    use_bo = bool(np.any(bo))
    use_bi = bool(np.any(bi))
    use_bout = bool(np.any(bout))
    use_mask = bool(np.any(mask))
    use_ln1 = bool(np.any(ln1_g != 1.0) or np.any(ln1_b))
    use_ln2 = bool(np.any(ln2_g != 1.0) or np.any(ln2_b))
    flags = (use_bq, use_bk, use_bv, use_bo, use_bi, use_bout,
             use_mask, use_ln1, use_ln2)

    nc = build_nc(flags)

    # per-expert packed weights, converted once and reused across cores.
    # QKV/Wo go as scaled fp8 (DoubleRow matmuls); FFN weights as bf16.
    packed = {}
    scales = {}
    for e in set(int(v) for v in eidx):
        s_wq = _absmax(Wq[e]) / F8MAX
        s_wk = _absmax(Wk[e]) / F8MAX
        s_wv = _absmax(Wv[e]) / F8MAX
        s_wo = _absmax(Wo[e]) / F8MAX
        scales[e] = (s_wq, s_wk, s_wv, s_wo)
        packed[e] = {
            "wq": _f8(_pack_lhsT(Wq[e], HK), s_wq),
            "wk": _f8(_pack_lhsT(Wk[e], HK), s_wk),
            "wv": _f8(Wv[e].reshape(HK, P, H), s_wv),
            "wo": _f8(Wo[e].reshape(HK, P, H), s_wo),
            "wi": _bf(_pack_lhsT(Wi[e], FK)),
            "wout": _bf(Wout[e].reshape(FK, P, H)),
        }

    in_maps = []
    for b in range(B):
        e = int(eidx[b])
        xb = hs[b]
        s_x = _absmax(xb) / F8MAX
        s_wq, s_wk, s_wv, s_wo = scales[e]
        im = {
            "x": _bf(xb),
            "xT": _f8(xb.T, s_x),
            "scl": np.array([s_x * s_wq, s_x * s_wk, s_x * s_wv,
                             s_wo / CTXS], np.float32),
        }
        im.update(packed[e])
        if use_bq:
            im["bq"] = np.ascontiguousarray(bq[e].reshape(HK, P).T)
        if use_bk:
            im["bk"] = np.ascontiguousarray(bk[e].reshape(HK, P).T)
        if use_bv:
            im["bv"] = bv[e]
        if use_bo:
            im["bo"] = bo[e]
        if use_bi:
            im["bi"] = np.ascontiguousarray(bi[e].reshape(FK, P).T)
        if use_bout:
            im["bout"] = bout[e]
        if use_mask:
            im["msk"] = np.ascontiguousarray(mask[b, 0, 0, :].reshape(SQ, P).T)
        if use_ln1:
            im["ln1g"] = ln1_g
            im["ln1b"] = ln1_b
        if use_ln2:
            im["ln2g"] = ln2_g
            im["ln2b"] = ln2_b
        in_maps.append(im)

    from concourse.bass_utils import run_bass_kernel_spmd
    res = run_bass_kernel_spmd(nc, in_maps, core_ids=list(range(N_CORES)),
                               **RUN_KWARGS)
    global LAST_RESULTS
    LAST_RESULTS = res
    out = np.stack([res.results[b]["out"] for b in range(B)], axis=0)
    return out.astype(np.float32)


RUN_KWARGS = {}
LAST_RESULTS = None


if __name__ == "__main__":
    rng = np.random.default_rng(0)
    demo = {
        "hidden_states": rng.standard_normal((8, S, H), dtype=np.float32),
        "expert_idx": rng.integers(0, 4, size=8).astype(np.int32),
        "attention_mask": np.zeros((8, 1, 1, S), np.float32),
        "Wq": 0.02 * rng.standard_normal((4, H, H), dtype=np.float32),
        "bq": np.zeros((4, H), np.float32),
        "Wk": 0.02 * rng.standard_normal((4, H, H), dtype=np.float32),
        "bk": np.zeros((4, H), np.float32),
        "Wv": 0.02 * rng.standard_normal((4, H, H), dtype=np.float32),
        "bv": np.zeros((4, H), np.float32),
        "Wo": 0.02 * rng.standard_normal((4, H, H), dtype=np.float32),
        "bo": np.zeros((4, H), np.float32),
        "ln1_g": np.ones((H,), np.float32),
        "ln1_b": np.zeros((H,), np.float32),
        "Wi": 0.02 * rng.standard_normal((4, H, FF), dtype=np.float32),
        "bi": np.zeros((4, FF), np.float32),
        "Wout": 0.02 * rng.standard_normal((4, FF, H), dtype=np.float32),
        "bout": np.zeros((4, H), np.float32),
        "ln2_g": np.ones((H,), np.float32),
        "ln2_b": np.zeros((H,), np.float32),
    }
    out = kernel(**demo)
    print("out", out.shape, out.dtype, float(np.abs(out).mean()))
